# revision 32
# baseline (speedup 1.0000x reference)
"""DiffFDN Trainium2 kernel, v9: SBUF-resident U-ring + per-line PE matmuls.

Per core (4 items): the 48000-step FDN scan becomes 115 blocks of L=416
columns. The recurrence state is the history of U = A@z (the per-step
delay-line inputs), kept in an SBUF ring [128, 9984+416] (f32r), plus a
DRAM copy of the long-delay rows for SWDGE gathers.

Line handling is latency-stratified by delay:
 - lines 0-5 (d=1009..1543, freshest reads): six per-line matmuls whose
   rhs reads the ring directly at the line's own shifted span.  Matmul
   cost depends only on output columns, so per-line contraction is free.
   Consecutive PE instructions alternate between two PSUM accumulation
   groups (blocks b, b+1) so the tensor engine never stalls on the
   same-PSUM WAW semaphore and keeps its clock ramped.
 - lines 6-8: per-line engine copies ring->S (4-row groups at the
   32-aligned partition bases 0/32/64; DVE and Act).
 - lines 9-15 (d>=1987): rows 96-123; one SWDGE indirect gather per
   block from DRAM h_d with a write fence KD=5 blocks back.  Line 10+
   is entirely covered by the fence; line 9's last 93 columns race it
   and are patched from the ring (partition base 96).
 - y rows 28-31 accumulate C@z in the same matmuls; chunks of 5 blocks
   are DMA'd from the ring straight to the output.
"""

import numpy as np

SR = 48000
IR_LEN = 48000
DELAYS = [1009, 1123, 1231, 1321, 1433, 1543, 1657, 1777, 1879, 1987,
          2081, 2179, 2287, 2383, 2503, 2617]
N = 16
BATCH = 32
NCORES = 8
IPC = BATCH // NCORES

L = 416                      # block columns
B0 = 2                       # first block with any nonzero output
NB = 115                     # last block index; blocks B0..NB cover IR_LEN
KD = 5                       # gather(b) fences on write(b-KD)
W = 9984                     # ring period (24 blocks)
MIR = 416                    # mirror tail so shifted reads never wrap
PAD = 2620                   # zero history before t=0 in DRAM
HD = PAD + (NB + 1) * L + 16  # DRAM h width
YROW = 28
N_PE = 6                     # lines 0..5 via per-line matmuls
DMA0 = 9                     # lines 9..15 via SWDGE gather, rows 96..123
P9C = KD * L - DELAYS[9]     # line-9 patched (raced) columns: 93
YCH = 5 * L                  # y output chunk (2080 cols)
NYCH = IR_LEN // YCH         # 23 full chunks, then a tail

_BUILT = None


def _outrow(i, j):
    """U/psum/ring partition row of (line i, item j)."""
    if i <= 5:
        return 4 + 4 * i + j
    if i == 6:
        return 0 + j
    if i == 7:
        return 32 + j
    if i == 8:
        return 64 + j
    return 96 + 4 * (i - DMA0) + j


def _expm64(M):
    M = M.astype(np.float64)
    nrm = np.linalg.norm(M, ord=np.inf)
    k = max(0, int(np.ceil(np.log2(max(nrm, 1e-30)))) + 2)
    Ms = M / (2.0 ** k)
    E = np.eye(M.shape[0]) + Ms
    term = Ms.copy()
    for i in range(2, 18):
        term = term @ Ms / i
        E = E + term
    for _ in range(k):
        E = E @ E
    return E


def _prologue(x, WA, bA, WB, bB, WC, bC):
    x = np.asarray(x, np.float32)
    feat = x.mean(axis=1)
    A = np.tanh(feat @ np.asarray(WA).T + bA).reshape(-1, N, N)
    Bv = np.tanh(feat @ np.asarray(WB).T + bB)
    Cv = np.tanh(feat @ np.asarray(WC).T + bC)
    S = np.triu(A, 1)
    S = S - np.swapaxes(S, -1, -2)
    g = 10.0 ** (-3.0 / SR)
    G = g ** np.asarray(DELAYS, np.float64)
    A_g = np.stack([_expm64(S[b]) for b in range(S.shape[0])])
    A_g = (A_g * G[None, None, :]).astype(np.float32)
    return A_g, Bv.astype(np.float32), Cv.astype(np.float32)


def _core_inputs(A_g4, Bv4, Cv4):
    """lhsT_S [128,128], lhsT_l [6,32,128], bv [128,1] for one core."""
    lhsT_S = np.zeros((128, 128), np.float32)
    for i in range(N_PE, N):
        for j in range(IPC):
            k = _outrow(i, j)
            for ip in range(N):
                lhsT_S[k, _outrow(ip, j)] = A_g4[j, ip, i]
            lhsT_S[k, YROW + j] = Cv4[j, i]
    lhsT_l = np.zeros((N_PE, 32, 128), np.float32)
    for ell in range(N_PE):
        for j in range(IPC):
            k = _outrow(ell, j)
            assert k < 32
            for ip in range(N):
                lhsT_l[ell, k, _outrow(ip, j)] = A_g4[j, ip, ell]
            lhsT_l[ell, k, YROW + j] = Cv4[j, ell]
    bv = np.zeros((128, 1), np.float32)
    for i in range(N):
        for j in range(IPC):
            bv[_outrow(i, j), 0] = Bv4[j, i]
    return lhsT_S, lhsT_l, bv


def _offsets():
    """offs[r, b] = flat h_d element index of (row r, col PAD+n0-d)."""
    offs = np.zeros((28, NB + 1), np.uint32)
    for b in range(B0, NB + 1):
        n0 = L * b
        for i in range(DMA0, N):
            for j in range(IPC):
                r = _outrow(i, j) - 96
                offs[r, b] = r * HD + (PAD + n0 - DELAYS[i])
    return offs


def _build():
    global _BUILT
    if _BUILT is not None:
        return _BUILT
    import concourse.bacc as bacc
    import concourse.bass as bass
    import concourse.mybir as mybir
    import concourse.tile as tile

    fp32 = mybir.dt.float32
    f32r = mybir.dt.float32r
    u32 = mybir.dt.uint32
    nc = bacc.Bacc("TRN2", target_bir_lowering=False, debug=False)
    lhsTS_d = nc.dram_tensor("lhsTS", [128, 128], f32r, kind="ExternalInput")
    lhsTl_d = nc.dram_tensor("lhsTl", [N_PE * 32, 128], f32r, kind="ExternalInput")
    bv_d = nc.dram_tensor("bv", [128, 1], f32r, kind="ExternalInput")
    offs_d = nc.dram_tensor("offs", [28, NB + 1], u32, kind="ExternalInput")
    y_d = nc.dram_tensor("y", [IPC, IR_LEN], f32r, kind="ExternalOutput")
    h_d = nc.dram_tensor("hist", [28, HD], f32r)

    d6, d7, d8, d9 = DELAYS[6], DELAYS[7], DELAYS[8], DELAYS[9]

    with tile.TileContext(nc) as tc:
        with tc.tile_pool(name="const", bufs=1) as cpool, \
             tc.tile_pool(name="ring", bufs=1) as rpool, \
             tc.tile_pool(name="sg", bufs=1) as spool, \
             tc.tile_pool(name="ps", bufs=6, space="PSUM") as ppool:
            lhsT_S = cpool.tile([128, 128], f32r)
            nc.sync.dma_start(lhsT_S[:, :], lhsTS_d[:, :])
            lhsT_l = [cpool.tile([32, 128], f32r, name=f"lhsTl{e}")
                      for e in range(N_PE)]
            for e in range(N_PE):
                nc.sync.dma_start(lhsT_l[e][:, :],
                                  lhsTl_d[e * 32:(e + 1) * 32, :])
            offs = cpool.tile([28, NB + 1], u32)
            nc.sync.dma_start(offs[:, :], offs_d[:, :])

            ring = rpool.tile([128, W + MIR], f32r)
            half = (W + MIR) // 2
            nc.vector.memset(ring[:, 0:half].bitcast(fp32), 0.0)
            nc.gpsimd.memset(ring[:, half:].bitcast(fp32), 0.0)
            # impulse U[:,0] = B at ring col 0 (and its mirror image)
            nc.sync.dma_start(ring[:, 0:1], bv_d[:, :])
            nc.sync.dma_start(ring[:, W:W + 1], bv_d[:, :])

            # DRAM history: zeros over [0, PAD+2L) incl. blocks 0-1, then
            # the impulse column at PAD.
            z = spool.tile([28, PAD + 2 * L], f32r)
            nc.gpsimd.memset(z[0:28, :].bitcast(fp32), 0.0)
            nc.scalar.dma_start(z[0:28, PAD:PAD + 1], bv_d[96:124, :])
            nc.scalar.dma_start(h_d[:, 0:PAD + 2 * L], z[0:28, :])

            # rotating S tiles, zeroed once (unwritten rows stay zero)
            NS = 7
            S_t = [spool.tile([128, L], f32r, name=f"S{k}")
                   for k in range(NS)]
            for s in S_t:
                nc.gpsimd.memset(s[:, :].bitcast(fp32), 0.0)

            def S(b):
                return S_t[b % NS]

            def pos(b):
                return (L * b) % W

            def rd(b, d, c0=0):
                """ring col of (block b col c0) shifted back by d."""
                return (L * b - d + c0) % W

            def emit_gather(b):
                # fence: for b>=B0+KD the in_ slice covers writes <= b-KD;
                # earlier blocks depend only on the zero/impulse prologue.
                end = PAD + (b - KD + 1) * L if b >= B0 + KD else PAD + 2 * L
                nc.gpsimd.indirect_dma_start(
                    out=S(b)[96:124, :], out_offset=None,
                    in_=h_d[0:28, 0:end],
                    in_offset=bass.IndirectOffsetOnAxis(
                        ap=offs[:, b:b + 1], axis=1),
                )

            def emit_line6(b):
                nc.vector.tensor_copy(
                    S(b)[0:4, :], ring[0:4, rd(b, d6):rd(b, d6) + L])

            def emit_line7(b):
                nc.scalar.copy(
                    S(b)[32:36, :], ring[32:36, rd(b, d7):rd(b, d7) + L])

            def emit_line8(b):
                nc.scalar.copy(
                    S(b)[64:68, :], ring[64:68, rd(b, d8):rd(b, d8) + L])

            def emit_patch9(b):
                c0 = L - P9C
                nc.vector.tensor_copy(
                    S(b)[96:100, c0:L],
                    ring[96:100, rd(b, d9, c0):rd(b, d9, c0) + P9C])

            def emit_write(b, ps):
                nc.sync.dma_start(
                    h_d[:, PAD + L * b:PAD + L * (b + 1)],
                    ring[96:124, pos(b):pos(b) + L])

            def emit_ychunk(k):
                # y cols [YCH*k, YCH*(k+1)) once block 5k+4 is in the ring
                c0 = YCH * k
                p = c0 % W
                if p + YCH <= W + MIR:
                    nc.sync.dma_start(
                        y_d[:, c0:c0 + YCH], ring[YROW:YROW + 4, p:p + YCH])
                else:
                    c1 = W - p
                    nc.sync.dma_start(
                        y_d[:, c0:c0 + c1], ring[YROW:YROW + 4, p:W])
                    nc.sync.dma_start(
                        y_d[:, c0 + c1:c0 + YCH],
                        ring[YROW:YROW + 4, 0:YCH - c1])

            # warm-up emissions whose target precedes the main-loop window
            for t in range(B0, B0 + KD):
                emit_gather(t)
            for t in range(B0, B0 + 3):
                emit_line6(t)
            for t in range(B0, B0 + 4):
                emit_line7(t)
                emit_line8(t)
                emit_patch9(t)

            ps_of = {}

            def emit_pe_group_pair(b1, b2):
                """Per-line + S matmuls for blocks b1, b2 interleaved so
                consecutive PE instructions hit different PSUM groups."""
                for b in (b1, b2):
                    if b is not None:
                        ps_of[b] = ppool.tile([128, L], fp32,
                                              name=f"ps{b}", tag="ps")
                steps = [("l", e) for e in range(N_PE - 1, -1, -1)] + [("S", None)]
                for kind, e in steps:
                    for b in (b1, b2):
                        if b is None:
                            continue
                        ps = ps_of[b]
                        first = kind == "l" and e == N_PE - 1
                        last = kind == "S"
                        if kind == "l":
                            d = DELAYS[e]
                            nc.tensor.matmul(
                                ps[:, :], lhsT_l[e][:, :],
                                ring[0:32, rd(b, d):rd(b, d) + L],
                                start=first, stop=last)
                        else:
                            nc.tensor.matmul(
                                ps[:, :], lhsT_S[:, :], S(b)[:, :],
                                start=first, stop=last)

            pairs = [(b, b + 1 if b + 1 <= NB else None)
                     for b in range(B0, NB + 1, 2)]
            for b1, b2 in pairs:
                emit_pe_group_pair(b1, b2)
                # ring copies get scheduler priority: they gate the next
                # blocks' line matmuls and the DRAM-write fence
                with tc.high_priority():
                    for b in (b1, b2):
                        if b is None:
                            continue
                        ps = ps_of[b]
                        nc.vector.tensor_copy(ring[:, pos(b):pos(b) + L],
                                              ps[:, 0:L])
                        if pos(b) == 0:
                            nc.vector.tensor_copy(ring[:, W:W + MIR],
                                                  ps[:, 0:L])
                for b in (b1, b2):
                    if b is None:
                        continue
                    ps = ps_of.pop(b)
                    emit_write(b, ps)
                    if b + 3 <= NB:
                        emit_line6(b + 3)
                    if b + 4 <= NB:
                        emit_line7(b + 4)
                        emit_line8(b + 4)
                        emit_patch9(b + 4)
                    if b + KD <= NB:
                        emit_gather(b + KD)
                    if b >= 4 and (b - 4) % 5 == 0 and (b - 4) // 5 < NYCH:
                        emit_ychunk((b - 4) // 5)

            # tail: y cols [47840, 48000) live at ring [7904, 8064)
            nc.sync.dma_start(y_d[:, NYCH * YCH:IR_LEN],
                              ring[YROW:YROW + 4, 7904:8064])
    nc.compile()
    _BUILT = nc
    return nc


def _in_maps(x, WA, bA, WB, bB, WC, bC):
    A_g, Bv, Cv = _prologue(x, WA, bA, WB, bB, WC, bC)
    offs = _offsets()
    in_maps = []
    for k in range(NCORES):
        sl = slice(k * IPC, (k + 1) * IPC)
        lhsT_S, lhsT_l, bv = _core_inputs(A_g[sl], Bv[sl], Cv[sl])
        in_maps.append({
            "lhsTS": lhsT_S,
            "lhsTl": lhsT_l.reshape(N_PE * 32, 128),
            "bv": bv, "offs": offs,
        })
    return in_maps


def kernel(x, WA, bA, WB, bB, WC, bC):
    from concourse import bass_utils

    in_maps = _in_maps(x, WA, bA, WB, bB, WC, bC)
    nc = _build()
    res = bass_utils.run_bass_kernel_spmd(nc, in_maps, core_ids=list(range(NCORES)))
    y = np.concatenate([res.results[k]["y"] for k in range(NCORES)], axis=0)
    return y[:, None, :].astype(np.float32)


# revision 34
# speedup vs baseline: 1.0680x; 1.0680x over previous
"""DiffFDN Trainium2 kernel, v9: SBUF-resident U-ring + per-line PE matmuls.

Per core (4 items): the 48000-step FDN scan becomes 115 blocks of L=416
columns. The recurrence state is the history of U = A@z (the per-step
delay-line inputs), kept in an SBUF ring [128, 9984+416] (f32r), plus a
DRAM copy of the long-delay rows for SWDGE gathers.

Line handling is latency-stratified by delay:
 - lines 0-5 (d=1009..1543, freshest reads): six per-line matmuls whose
   rhs reads the ring directly at the line's own shifted span.  Matmul
   cost depends only on output columns, so per-line contraction is free.
   Consecutive PE instructions alternate between two PSUM accumulation
   groups (blocks b, b+1) so the tensor engine never stalls on the
   same-PSUM WAW semaphore and keeps its clock ramped.
 - lines 6-8: per-line engine copies ring->S (4-row groups at the
   32-aligned partition bases 0/32/64; DVE and Act).
 - lines 9-15 (d>=1987): rows 96-123; one SWDGE indirect gather per
   block from DRAM h_d with a write fence KD=5 blocks back.  Line 10+
   is entirely covered by the fence; line 9's last 93 columns race it
   and are patched from the ring (partition base 96).
 - y rows 28-31 accumulate C@z in the same matmuls; chunks of 5 blocks
   are DMA'd from the ring straight to the output.
"""

import numpy as np

SR = 48000
IR_LEN = 48000
DELAYS = [1009, 1123, 1231, 1321, 1433, 1543, 1657, 1777, 1879, 1987,
          2081, 2179, 2287, 2383, 2503, 2617]
N = 16
BATCH = 32
NCORES = 8
IPC = BATCH // NCORES

L = 344                      # block columns
B0 = 2                       # first block with any nonzero output
NB = 139                     # last block index; blocks B0..NB cover IR_LEN
KD = 6                       # gather(b) fences on write(b-KD)
W = 10320                    # ring period (30 blocks)
MIR = 344                    # mirror tail so shifted reads never wrap
PAD = 2620                   # zero history before t=0 in DRAM
HD = PAD + (NB + 1) * L + 16  # DRAM h width
YROW = 28
N_PE = 6                     # lines 0..5 via per-line matmuls
DMA0 = 10                    # lines 10..15 via SWDGE gather, rows 100..123

YCH = 5 * L                  # y output chunk (2080 cols)
NYCH = IR_LEN // YCH         # 27 full chunks, then a tail

_BUILT = None


def _outrow(i, j):
    """U/psum/ring partition row of (line i, item j)."""
    if i <= 5:
        return 4 + 4 * i + j
    if i == 6:
        return 0 + j
    if i == 7:
        return 32 + j
    if i == 8:
        return 64 + j
    if i == 9:
        return 96 + j
    return 100 + 4 * (i - DMA0) + j


def _expm64(M):
    M = M.astype(np.float64)
    nrm = np.linalg.norm(M, ord=np.inf)
    k = max(0, int(np.ceil(np.log2(max(nrm, 1e-30)))) + 2)
    Ms = M / (2.0 ** k)
    E = np.eye(M.shape[0]) + Ms
    term = Ms.copy()
    for i in range(2, 18):
        term = term @ Ms / i
        E = E + term
    for _ in range(k):
        E = E @ E
    return E


def _prologue(x, WA, bA, WB, bB, WC, bC):
    x = np.asarray(x, np.float32)
    feat = x.mean(axis=1)
    A = np.tanh(feat @ np.asarray(WA).T + bA).reshape(-1, N, N)
    Bv = np.tanh(feat @ np.asarray(WB).T + bB)
    Cv = np.tanh(feat @ np.asarray(WC).T + bC)
    S = np.triu(A, 1)
    S = S - np.swapaxes(S, -1, -2)
    g = 10.0 ** (-3.0 / SR)
    G = g ** np.asarray(DELAYS, np.float64)
    A_g = np.stack([_expm64(S[b]) for b in range(S.shape[0])])
    A_g = (A_g * G[None, None, :]).astype(np.float32)
    return A_g, Bv.astype(np.float32), Cv.astype(np.float32)


def _core_inputs(A_g4, Bv4, Cv4):
    """lhsT_S [128,128], lhsT_l [6,32,128], bv [128,1] for one core."""
    lhsT_S = np.zeros((128, 128), np.float32)
    for i in range(N_PE, N):
        for j in range(IPC):
            k = _outrow(i, j)
            for ip in range(N):
                lhsT_S[k, _outrow(ip, j)] = A_g4[j, ip, i]
            lhsT_S[k, YROW + j] = Cv4[j, i]
    lhsT_l = np.zeros((N_PE, 32, 128), np.float32)
    for ell in range(N_PE):
        for j in range(IPC):
            k = _outrow(ell, j)
            assert k < 32
            for ip in range(N):
                lhsT_l[ell, k, _outrow(ip, j)] = A_g4[j, ip, ell]
            lhsT_l[ell, k, YROW + j] = Cv4[j, ell]
    bv = np.zeros((128, 1), np.float32)
    for i in range(N):
        for j in range(IPC):
            bv[_outrow(i, j), 0] = Bv4[j, i]
    return lhsT_S, lhsT_l, bv


def _offsets():
    """offs[r, b] = flat h_d element index of (row r, col PAD+n0-d)."""
    offs = np.zeros((24, NB + 1), np.uint32)
    for b in range(B0, NB + 1):
        n0 = L * b
        for i in range(DMA0, N):
            for j in range(IPC):
                r = _outrow(i, j) - 100
                offs[r, b] = r * HD + (PAD + n0 - DELAYS[i])
    return offs


def _build():
    global _BUILT
    if _BUILT is not None:
        return _BUILT
    import concourse.bacc as bacc
    import concourse.bass as bass
    import concourse.mybir as mybir
    import concourse.tile as tile

    fp32 = mybir.dt.float32
    f32r = mybir.dt.float32r
    u32 = mybir.dt.uint32
    nc = bacc.Bacc("TRN2", target_bir_lowering=False, debug=False)
    lhsTS_d = nc.dram_tensor("lhsTS", [128, 128], f32r, kind="ExternalInput")
    lhsTl_d = nc.dram_tensor("lhsTl", [N_PE * 32, 128], f32r, kind="ExternalInput")
    bv_d = nc.dram_tensor("bv", [128, 1], f32r, kind="ExternalInput")
    offs_d = nc.dram_tensor("offs", [24, NB + 1], u32, kind="ExternalInput")
    y_d = nc.dram_tensor("y", [IPC, IR_LEN], f32r, kind="ExternalOutput")
    h_d = nc.dram_tensor("hist", [24, HD], f32r)

    d6, d7, d8, d9 = DELAYS[6], DELAYS[7], DELAYS[8], DELAYS[9]

    with tile.TileContext(nc) as tc:
        with tc.tile_pool(name="const", bufs=1) as cpool, \
             tc.tile_pool(name="ring", bufs=1) as rpool, \
             tc.tile_pool(name="sg", bufs=1) as spool, \
             tc.tile_pool(name="ps", bufs=6, space="PSUM") as ppool:
            lhsT_S = cpool.tile([128, 128], f32r)
            nc.sync.dma_start(lhsT_S[:, :], lhsTS_d[:, :])
            lhsT_l = [cpool.tile([32, 128], f32r, name=f"lhsTl{e}")
                      for e in range(N_PE)]
            for e in range(N_PE):
                nc.sync.dma_start(lhsT_l[e][:, :],
                                  lhsTl_d[e * 32:(e + 1) * 32, :])
            offs = cpool.tile([24, NB + 1], u32)
            nc.sync.dma_start(offs[:, :], offs_d[:, :])

            ring = rpool.tile([128, W + MIR], f32r)
            half = (W + MIR) // 2
            nc.vector.memset(ring[:, 0:half].bitcast(fp32), 0.0)
            nc.gpsimd.memset(ring[:, half:].bitcast(fp32), 0.0)
            # impulse U[:,0] = B at ring col 0 (and its mirror image)
            nc.sync.dma_start(ring[:, 0:1], bv_d[:, :])
            nc.sync.dma_start(ring[:, W:W + 1], bv_d[:, :])

            # DRAM history: zeros over [0, PAD+2L) incl. blocks 0-1, then
            # the impulse column at PAD.
            z = spool.tile([24, PAD + 2 * L], f32r)
            nc.gpsimd.memset(z[0:24, :].bitcast(fp32), 0.0)
            nc.scalar.dma_start(z[0:24, PAD:PAD + 1], bv_d[100:124, :])
            nc.scalar.dma_start(h_d[:, 0:PAD + 2 * L], z[0:24, :])

            # rotating S tiles, zeroed once (unwritten rows stay zero)
            NS = 8
            S_t = [spool.tile([128, L], f32r, name=f"S{k}")
                   for k in range(NS)]
            for s in S_t:
                nc.gpsimd.memset(s[:, :].bitcast(fp32), 0.0)

            def S(b):
                return S_t[b % NS]

            def pos(b):
                return (L * b) % W

            def rd(b, d, c0=0):
                """ring col of (block b col c0) shifted back by d."""
                return (L * b - d + c0) % W

            def emit_gather(b):
                # fence: for b>=B0+KD the in_ slice covers writes <= b-KD;
                # earlier blocks depend only on the zero/impulse prologue.
                end = PAD + (b - KD + 1) * L if b >= B0 + KD else PAD + 2 * L
                nc.gpsimd.indirect_dma_start(
                    out=S(b)[100:124, :], out_offset=None,
                    in_=h_d[0:24, 0:end],
                    in_offset=bass.IndirectOffsetOnAxis(
                        ap=offs[:, b:b + 1], axis=1),
                )

            def emit_line6(b):
                nc.vector.tensor_copy(
                    S(b)[0:4, :], ring[0:4, rd(b, d6):rd(b, d6) + L])

            def emit_line7(b):
                nc.scalar.copy(
                    S(b)[32:36, :], ring[32:36, rd(b, d7):rd(b, d7) + L])

            def emit_line8(b):
                nc.scalar.copy(
                    S(b)[64:68, :], ring[64:68, rd(b, d8):rd(b, d8) + L])

            def emit_line9(b):
                nc.vector.tensor_copy(
                    S(b)[96:100, :], ring[96:100, rd(b, d9):rd(b, d9) + L])

            def emit_write(b, ps):
                nc.sync.dma_start(
                    h_d[:, PAD + L * b:PAD + L * (b + 1)],
                    ring[100:124, pos(b):pos(b) + L])

            def emit_ychunk(k):
                # y cols [YCH*k, YCH*(k+1)) once block 5k+4 is in the ring
                c0 = YCH * k
                p = c0 % W
                if p + YCH <= W + MIR:
                    nc.sync.dma_start(
                        y_d[:, c0:c0 + YCH], ring[YROW:YROW + 4, p:p + YCH])
                else:
                    c1 = W - p
                    nc.sync.dma_start(
                        y_d[:, c0:c0 + c1], ring[YROW:YROW + 4, p:W])
                    nc.sync.dma_start(
                        y_d[:, c0 + c1:c0 + YCH],
                        ring[YROW:YROW + 4, 0:YCH - c1])

            # warm-up emissions whose target precedes the main-loop window
            for t in range(B0, B0 + KD):
                emit_gather(t)
            for t in range(B0, B0 + 4):
                emit_line6(t)
            for t in range(B0, B0 + 5):
                emit_line7(t)
                emit_line8(t)
                emit_line9(t)

            ps_of = {}

            def emit_pe_group_pair(b1, b2):
                """Per-line + S matmuls for blocks b1, b2 interleaved so
                consecutive PE instructions hit different PSUM groups."""
                for b in (b1, b2):
                    if b is not None:
                        ps_of[b] = ppool.tile([128, L], fp32,
                                              name=f"ps{b}", tag="ps")
                steps = [("l", e) for e in range(N_PE - 1, -1, -1)] + [("S", None)]
                for kind, e in steps:
                    for b in (b1, b2):
                        if b is None:
                            continue
                        ps = ps_of[b]
                        first = kind == "l" and e == N_PE - 1
                        last = kind == "S"
                        if kind == "l":
                            d = DELAYS[e]
                            nc.tensor.matmul(
                                ps[:, :], lhsT_l[e][:, :],
                                ring[0:32, rd(b, d):rd(b, d) + L],
                                start=first, stop=last)
                        else:
                            nc.tensor.matmul(
                                ps[:, :], lhsT_S[:, :], S(b)[:, :],
                                start=first, stop=last)

            pairs = [(b, b + 1 if b + 1 <= NB else None)
                     for b in range(B0, NB + 1, 2)]
            for b1, b2 in pairs:
                emit_pe_group_pair(b1, b2)
                # ring copies get scheduler priority: they gate the next
                # blocks' line matmuls and the DRAM-write fence
                with tc.high_priority():
                    for b in (b1, b2):
                        if b is None:
                            continue
                        ps = ps_of[b]
                        nc.vector.tensor_copy(ring[:, pos(b):pos(b) + L],
                                              ps[:, 0:L])
                        if pos(b) == 0:
                            nc.vector.tensor_copy(ring[:, W:W + MIR],
                                                  ps[:, 0:L])
                for b in (b1, b2):
                    if b is None:
                        continue
                    ps = ps_of.pop(b)
                    emit_write(b, ps)
                    if b + 4 <= NB:
                        emit_line6(b + 4)
                    if b + 5 <= NB:
                        emit_line7(b + 5)
                        emit_line8(b + 5)
                        emit_line9(b + 5)
                    if b + KD <= NB:
                        emit_gather(b + KD)
                    if b >= 4 and (b - 4) % 5 == 0 and (b - 4) // 5 < NYCH:
                        emit_ychunk((b - 4) // 5)

            # tail: y cols [46440, 48000) live at ring [5160, 6720)
            nc.sync.dma_start(y_d[:, NYCH * YCH:IR_LEN],
                              ring[YROW:YROW + 4, 5160:6720])
    nc.compile()
    _BUILT = nc
    return nc


def _in_maps(x, WA, bA, WB, bB, WC, bC):
    A_g, Bv, Cv = _prologue(x, WA, bA, WB, bB, WC, bC)
    offs = _offsets()
    in_maps = []
    for k in range(NCORES):
        sl = slice(k * IPC, (k + 1) * IPC)
        lhsT_S, lhsT_l, bv = _core_inputs(A_g[sl], Bv[sl], Cv[sl])
        in_maps.append({
            "lhsTS": lhsT_S,
            "lhsTl": lhsT_l.reshape(N_PE * 32, 128),
            "bv": bv, "offs": offs,
        })
    return in_maps


def kernel(x, WA, bA, WB, bB, WC, bC):
    from concourse import bass_utils

    in_maps = _in_maps(x, WA, bA, WB, bB, WC, bC)
    nc = _build()
    res = bass_utils.run_bass_kernel_spmd(nc, in_maps, core_ids=list(range(NCORES)))
    y = np.concatenate([res.results[k]["y"] for k in range(NCORES)], axis=0)
    return y[:, None, :].astype(np.float32)
